# revision 15
# baseline (speedup 1.0000x reference)
"""Trainium2 Bass kernel for nn_Attention_46780783788294.

Multi-head causal-ish attention (mask fills with 0.0, not -inf) for
x:[2,2048,1024], 16 heads of d_head=64, fp32 in/out, bf16 compute.

Sharding: 8 cores = 2 batches x 4 head-groups (4 heads each). Each core
computes its batch/head-group partial output [2048,1024] (bf16); host sums
the 4 partials per batch in fp32 and adds b_O.

Per-core device program (all-transposed "S^T" layout, bf16 matmuls with
fp32 PSUM accumulation):
  xT_aug [1025,2048] (x^T plus ones row) and packed/augmented weights come
  from the host. QT/KT computed per head-pair [128,2048] (d on partitions);
  V computed in natural [k,d] layout [128,260] per k-block with a per-head
  ones column (from the bias-row trick) so the AV matmul accumulates the
  softmax denominator for free. Scores are built k-on-partitions so
  P = exp(mask * s / 8) feeds the AV matmul directly with no transposes;
  masked entries give exp(0)=1 exactly as the reference's 0.0-fill softmax
  requires. Fully-masked k-blocks are skipped: their contribution (suffix
  column-sums of V) is added analytically via ones-rhs matmuls. The two
  heads of a pair share one [128,1024] scores psum so each mask-mul/exp
  covers both heads in one instruction. Chunk-outer loop: each q-chunk is
  attended, divided, output-projected and DMA'd out before the next, so
  PE/ACT/DVE/DMA stay overlapped throughout.
"""

import os
import sys

import numpy as np


def _ensure_concourse():
    try:
        import concourse  # noqa: F401
    except ImportError:
        for p in ("/root/.axon_site", "/root/.axon_site/_ro/trn_rl_repo",
                  "/root/.axon_site/_ro/pypackages", "/opt/trn_rl_repo"):
            if os.path.isdir(p) and p not in sys.path:
                sys.path.append(p)


_ensure_concourse()

import concourse.bass as bass  # noqa: E402
import concourse.tile as tile  # noqa: E402
from concourse import bacc, mybir  # noqa: E402
from concourse import bass_utils  # noqa: E402
from contextlib import ExitStack  # noqa: E402

F32 = mybir.dt.float32
BF16 = mybir.dt.bfloat16
EXP = mybir.ActivationFunctionType.Exp

S = 2048      # sequence length
M = 1024      # d_model
DH = 64       # d_head
HL = 4        # heads per core
NP = 2        # head pairs per core
CH = 512      # q-chunk width
NCH = S // CH     # 4 q chunks
KB = S // 128     # 16 k blocks
MB = M // 128     # 8 m blocks
N_CORES = 8


def _emit(tc, nc, d, zero_bias):
    mm = nc.tensor.matmul
    with ExitStack() as ctx:
        # ---- persistent pools ----
        qkp = ctx.enter_context(tc.tile_pool(name="qkp", bufs=1))
        vp = ctx.enter_context(tc.tile_pool(name="vp", bufs=1))
        wop = ctx.enter_context(tc.tile_pool(name="wop", bufs=1))
        cst = ctx.enter_context(tc.tile_pool(name="cst", bufs=1))
        dnp = ctx.enter_context(tc.tile_pool(name="dnp", bufs=1))
        z2p = ctx.enter_context(tc.tile_pool(name="z2p", bufs=1))
        xp = ctx.enter_context(tc.tile_pool(name="xp", bufs=1))
        wp = ctx.enter_context(tc.tile_pool(name="wp", bufs=1))
        pp = ctx.enter_context(tc.tile_pool(name="pp", bufs=3))
        stg = ctx.enter_context(tc.tile_pool(name="stg", bufs=2))
        op_sb = ctx.enter_context(tc.tile_pool(name="op_sb", bufs=3))
        # PSUM budget is 8 banks, statically split: sps 2x2, zps 2x1, and a
        # single 2-slot pool shared by every 1-bank psum use
        psX = ctx.enter_context(tc.tile_pool(name="psX", bufs=2, space="PSUM"))
        psS = ctx.enter_context(tc.tile_pool(name="psS", bufs=2, space="PSUM"))
        psZ = ctx.enter_context(tc.tile_pool(name="psZ", bufs=1, space="PSUM"))

        qt = [qkp.tile([128, S], BF16, name=f"qt{p}") for p in range(NP)]
        kt = [qkp.tile([128, S], BF16, name=f"kt{p}") for p in range(NP)]
        vt = [vp.tile([128, 260], BF16, name=f"vt{j}") for j in range(KB)]
        wo_t = [wop.tile([128, M], BF16, name=f"wo{p}") for p in range(NP)]
        dtri = cst.tile([128, 256], BF16, name="dtri")
        e2_t = cst.tile([2, 128], BF16, name="e2")
        ones_row = cst.tile([1, CH], BF16, name="ones_row")
        ones_col = cst.tile([128, 1], BF16, name="ones_col")
        dnc = [[dnp.tile([2, CH], F32, name=f"dnc{c}_{p}")
                for p in range(NP)] for c in range(NCH)]
        z2u = [z2p.tile([128, S], BF16, name=f"z2u{p}") for p in range(NP)]
        sfx = [dnp.tile([1, 260], BF16, name=f"sfx{c}") for c in range(3)]
        b_sb = {j: dnp.tile([1, 260], BF16, name=f"bsb{j}") for j in range(KB)
                if j % 4}

        xt_t = [xp.tile([128, S], BF16, name=f"xt{mb}") for mb in range(MB)]
        wq_t = [wp.tile([128, 256], BF16, name=f"wq{mb}") for mb in range(MB)]
        wk_t = [wp.tile([128, 256], BF16, name=f"wk{mb}") for mb in range(MB)]
        wv_t = [wp.tile([128, 260], BF16, name=f"wv{mb}") for mb in range(MB)]
        if not zero_bias:
            xt_ones = xp.tile([1, S], BF16, name="xt_ones")
            wq_b = wp.tile([1, 256], BF16, name="wq_b")
            wk_b = wp.tile([1, 256], BF16, name="wk_b")
            wv_b = wp.tile([1, 260], BF16, name="wv_b")

        # DMA order: what attention chunk 0 needs first (wq/wk, x^T chunk 0,
        # wv, masks), then the rest of x^T; W_O last. Intro loads are split
        # across both HWDGE dispatch engines (sync + scalar) since the
        # scalar engine is idle until the first exp.
        for mb in range(MB):
            ea = nc.scalar if mb % 2 else nc.sync
            eb = nc.sync if mb % 2 else nc.scalar
            ea.dma_start(wq_t[mb][:], d["wq"][128 * mb:128 * (mb + 1), :])
            eb.dma_start(xt_t[mb][:, 0:CH],
                         d["xt"][128 * mb:128 * (mb + 1), 0:CH])
            ea.dma_start(wk_t[mb][:], d["wk"][128 * mb:128 * (mb + 1), :])
        for mb in range(MB):
            eng = nc.sync if mb % 2 else nc.scalar
            eng.dma_start(wv_t[mb][:], d["wv"][128 * mb:128 * (mb + 1), :])
        if not zero_bias:
            nc.sync.dma_start(wq_b[:], d["wq"][1024:1025, :])
            nc.sync.dma_start(wk_b[:], d["wk"][1024:1025, :])
            nc.sync.dma_start(wv_b[:], d["wv"][1024:1025, :])
            nc.sync.dma_start(xt_ones[:], d["xt"][1024:1025, :])
        nc.scalar.dma_start(dtri[:], d["mk"][:])
        nc.sync.dma_start(ones_row[:], d["cst"][0:1, :])
        nc.sync.dma_start(ones_col[:], d["cst"][0:128, 0:1])
        nc.sync.dma_start(e2_t[:], d["e2"][:])
        for c in range(1, NCH):
            for mb in range(MB):
                nc.sync.dma_start(
                    xt_t[mb][:, CH * c:CH * (c + 1)],
                    d["xt"][128 * mb:128 * (mb + 1), CH * c:CH * (c + 1)])
        for p in range(NP):
            nc.sync.dma_start(wo_t[p][:], d["wo"][128 * p:128 * (p + 1), :])

        def emit_v(j):
            ps = psX.tile([128, 260], F32, name="psv", tag="px")
            for mb in range(MB):
                mm(ps[:], xt_t[mb][:, 128 * j:128 * (j + 1)],
                   wv_t[mb][:], start=(mb == 0),
                   stop=(zero_bias and mb == MB - 1))
            if not zero_bias:
                mm(ps[:], xt_ones[:, 128 * j:128 * (j + 1)], wv_b[:],
                   start=False, stop=True)
            nc.scalar.copy(vt[j][:], ps[:])
            if zero_bias:
                # ones columns (the bias-row trick needs them even with zero
                # biases): set cols 64,129,194,259 to 1.0
                oc = vt[j].rearrange("p (h c) -> p h c", c=65)[:, :, 64]
                nc.gpsimd.memset(oc, 1.0)
            if j % 4:
                # per-block column sums of V_aug, for the fully-masked
                # column prefix of diagonal blocks
                bs = psX.tile([1, 260], F32, name="psb", tag="px")
                mm(bs[:], ones_col[:], vt[j][:], start=True, stop=True)
                nc.scalar.copy(b_sb[j][:], bs[:])

        def emit_qk(p, which, c):
            dst, wt = (qt, wq_t) if which == 0 else (kt, wk_t)
            ps = psX.tile([128, CH], F32, name="psqk", tag="px")
            for mb in range(MB):
                mm(ps[:], wt[mb][:, 128 * p:128 * (p + 1)],
                   xt_t[mb][:, CH * c:CH * (c + 1)],
                   start=(mb == 0), stop=(zero_bias and mb == MB - 1))
            if not zero_bias:
                wb = wq_b if which == 0 else wk_b
                mm(ps[:], wb[:, 128 * p:128 * (p + 1)],
                   xt_ones[:, CH * c:CH * (c + 1)], start=False, stop=True)
            nc.vector.tensor_copy(dst[p][:, CH * c:CH * (c + 1)], ps[:])

        zps_by_chunk = {}

        def emit_attn_head(ch):
            # scores/mask/exp/AV for all k-blocks of chunk ch (both pairs)
            nj = 4 * ch + 4
            zl = []
            for p in range(NP):
                h0, h1 = 2 * p, 2 * p + 1
                zps = [psZ.tile([65, CH], F32, name=f"zps{half}",
                                tag=f"zps{half}") for half in range(2)]
                zl.append(zps)
                for j in range(nj):
                    # both heads of the pair share one [128,1024] scores
                    # psum: one mask-mul + one exp per j. For diagonal
                    # blocks only the unmasked column suffix [w0:512) is
                    # computed; the fully-masked prefix contributes
                    # exp(0)=1 per element, added analytically from the
                    # block's V column sums.
                    r = j - 4 * ch
                    w0 = 128 * r if r > 0 else 0
                    n = CH - w0
                    last = ch == 3 and j == nj - 1
                    sps = psS.tile([128, 2 * CH], F32, name="sps", tag="sps")
                    mm(sps[:, w0:CH],
                       kt[p][0:64, 128 * j:128 * (j + 1)],
                       qt[p][0:64, CH * ch + w0:CH * (ch + 1)],
                       start=True, stop=True)
                    mm(sps[:, CH + w0:2 * CH],
                       kt[p][64:128, 128 * j:128 * (j + 1)],
                       qt[p][64:128, CH * ch + w0:CH * (ch + 1)],
                       start=True, stop=True)
                    sps3 = sps.rearrange("p (t c) -> p t c", t=2)
                    pt = pp.tile([128, 2 * CH], BF16, name="pt", tag="pt")
                    pt3 = pt.rearrange("p (t c) -> p t c", t=2)
                    if r >= 0:
                        # triangular mask on the 128-wide diagonal strip of
                        # both heads at once
                        strip = sps3[:, :, w0:w0 + 128]
                        dtri3 = dtri.rearrange("p (t c) -> p t c", t=2)
                        nc.vector.tensor_mul(strip, strip, dtri3)
                    if w0:
                        nc.scalar.activation(pt3[:, :, w0:CH],
                                             sps3[:, :, w0:CH], EXP,
                                             scale=0.125)
                    else:
                        nc.scalar.activation(pt[:], sps[:], EXP, scale=0.125)
                    mm(zps[0][:, w0:CH], vt[j][:, 65 * h0:65 * h0 + 65],
                       pt[:, w0:CH], start=(j == 0),
                       stop=(last and not w0))
                    mm(zps[1][:, w0:CH], vt[j][:, 65 * h1:65 * h1 + 65],
                       pt[:, CH + w0:2 * CH], start=(j == 0),
                       stop=(last and not w0))
                    if w0:
                        mm(zps[0][:, 0:w0], b_sb[j][:, 65 * h0:65 * h0 + 65],
                           ones_row[:, 0:w0], start=False, stop=last)
                        mm(zps[1][:, 0:w0], b_sb[j][:, 65 * h1:65 * h1 + 65],
                           ones_row[:, 0:w0], start=False, stop=last)
            zps_by_chunk[ch] = zl

        def emit_attn_zrel(ch):
            # suffix contribution + denominator/z extraction for chunk ch —
            # releases the z psum slots as early as possible
            zl = zps_by_chunk.pop(ch)
            for p in range(NP):
                for half in range(2):
                    h = 2 * p + half
                    hb = 64 * half
                    if ch < 3:
                        mm(zl[p][half][:], sfx[ch][:, 65 * h:65 * h + 65],
                           ones_row[:], start=False, stop=True)
                    # stage denom row at partition 0, DMA-scatter into
                    # dnc[ch] (compute-engine APs need 32-aligned base
                    # partitions; DMA APs don't)
                    dnst = pp.tile([1, CH], F32, name="dnst", tag="dnst",
                                   bufs=4)
                    nc.vector.tensor_copy(dnst[:], zl[p][half][64:65, :])
                    nc.sync.dma_start(dnc[ch][p][half:half + 1, :], dnst[:])
                    nc.vector.tensor_copy(
                        z2u[p][hb:hb + 64, CH * ch:CH * (ch + 1)],
                        zl[p][half][0:64, :])

        def emit_divE(ch):
            # divide chunk ch's z by the softmax denominators, project to
            # the output and stream to DRAM
            for p in range(NP):
                rdc = stg.tile([2, CH], F32, name="rdc", tag="rdc")
                nc.vector.reciprocal_approx_fast(rdc[:], dnc[ch][p][:])
                rdcb = stg.tile([2, CH], BF16, name="rdcb", tag="rdcb")
                nc.vector.tensor_copy(rdcb[:], rdc[:])
                bc = psX.tile([128, CH], F32, name="bc", tag="px")
                mm(bc[:], e2_t[:], rdcb[:], start=True, stop=True)
                nc.vector.tensor_mul(
                    z2u[p][:, CH * ch:CH * (ch + 1)],
                    z2u[p][:, CH * ch:CH * (ch + 1)], bc[:])

            for q in range(4 * ch, 4 * ch + 4):
                for mc in range(2):
                    ops = psX.tile([128, CH], F32, name="ops", tag="px")
                    for p in range(NP):
                        mm(ops[:], z2u[p][:, 128 * q:128 * (q + 1)],
                           wo_t[p][:, CH * mc:CH * (mc + 1)],
                           start=(p == 0), stop=(p == 1))
                    osb = op_sb.tile([128, CH], BF16, name="osb", tag="osb")
                    nc.vector.tensor_copy(osb[:], ops[:])
                    nc.sync.dma_start(
                        d["out"][128 * q:128 * (q + 1), CH * mc:CH * (mc + 1)],
                        osb[:])

        # ---- emission: interleave projections, attention, division and
        # output so every engine has work throughout. Chunk c's division +
        # output projection is emitted after chunk c+1's attention head so
        # the PE never blocks on the (latency-heavy) division chain. ----
        for p in range(NP):
            emit_qk(p, 0, 0)
            emit_qk(p, 1, 0)
        for j in range(4):
            emit_v(j)
        emit_attn_head(0)
        for j in range(4, KB):
            emit_v(j)
        for p in range(NP):
            for c in range(1, NCH):
                emit_qk(p, 0, c)
                emit_qk(p, 1, c)
        # suffix column-sums of V_aug for fully-masked regions
        for c in range(3):
            ps = psX.tile([1, 260], F32, name="pssfx", tag="px")
            for j in range(4 * c + 4, KB):
                mm(ps[:], ones_col[:], vt[j][:],
                   start=(j == 4 * c + 4), stop=(j == KB - 1))
            nc.vector.tensor_copy(sfx[c][:], ps[:])
        emit_attn_zrel(0)
        for ch in range(1, NCH):
            emit_attn_head(ch)
            emit_divE(ch - 1)
            emit_attn_zrel(ch)
        emit_divE(3)


def build_program(zero_bias=False):
    nc = bacc.Bacc("TRN2", target_bir_lowering=False, debug=False,
                   num_devices=N_CORES)
    d = {
        "xt": nc.dram_tensor("xt", [1025, S], BF16, kind="ExternalInput").ap(),
        "wq": nc.dram_tensor("wq", [1025, 256], BF16, kind="ExternalInput").ap(),
        "wk": nc.dram_tensor("wk", [1025, 256], BF16, kind="ExternalInput").ap(),
        "wv": nc.dram_tensor("wv", [1025, 260], BF16, kind="ExternalInput").ap(),
        "wo": nc.dram_tensor("wo", [256, M], BF16, kind="ExternalInput").ap(),
        "mk": nc.dram_tensor("mk", [128, 256], BF16, kind="ExternalInput").ap(),
        "e2": nc.dram_tensor("e2", [2, 128], BF16, kind="ExternalInput").ap(),
        "cst": nc.dram_tensor("cst", [128, CH], BF16, kind="ExternalInput").ap(),
        "out": nc.dram_tensor("out", [S, M], BF16, kind="ExternalOutput").ap(),
    }
    with tile.TileContext(nc) as tc:
        _emit(tc, nc, d, zero_bias)
    nc.compile()
    return nc


_CACHE = {}


def _get_program(zero_bias=False):
    key = ("nc", zero_bias)
    if key not in _CACHE:
        _CACHE[key] = build_program(zero_bias)
    return _CACHE[key]


def _pack_qk(w4, b4):
    # w4 [4,1024,64], b4 [4,64] -> [1025, 256] (m-major, head-major cols)
    r = np.empty((1025, 256), np.float32)
    r[:1024] = w4.transpose(1, 0, 2).reshape(1024, 256)
    r[1024] = b4.reshape(256)
    return r


def _pack_v(w4, b4):
    # [1025, 260]: per head 64 W_V cols + a ones-generating column
    r = np.zeros((1025, 260), np.float32)
    for h in range(4):
        r[:1024, 65 * h:65 * h + 64] = w4[h]
        r[1024, 65 * h:65 * h + 64] = b4[h]
        r[1024, 65 * h + 64] = 1.0
    return r


def prepare_in_maps(normalized_resid_pre, W_Q, b_Q, W_K, b_K, W_V, b_V, W_O,
                    b_O):
    import ml_dtypes
    bf16 = ml_dtypes.bfloat16
    x = np.asarray(normalized_resid_pre, np.float32)
    W_Q = np.asarray(W_Q, np.float32)
    b_Q = np.asarray(b_Q, np.float32)
    W_K = np.asarray(W_K, np.float32)
    b_K = np.asarray(b_K, np.float32)
    W_V = np.asarray(W_V, np.float32)
    b_V = np.asarray(b_V, np.float32)
    W_O = np.asarray(W_O, np.float32)

    tri = np.triu(np.ones((128, 128), np.float32))  # [k,q]: 1 where k <= q
    mk = np.tile(tri, (1, 2))  # both heads of a pair side by side
    e2 = np.zeros((2, 128), np.float32)
    e2[0, :64] = 1.0
    e2[1, 64:] = 1.0
    cstv = np.ones((128, CH), np.float32)

    xts = []
    for b in range(2):
        xt = np.empty((1025, S), np.float32)
        xt[:1024] = x[b].T
        xt[1024] = 1.0
        xts.append(xt.astype(bf16))

    in_maps = []
    for c in range(N_CORES):
        b, g = divmod(c, 4)
        hs = slice(4 * g, 4 * g + 4)
        in_maps.append({
            "xt": xts[b],
            "wq": _pack_qk(W_Q[hs], b_Q[hs]).astype(bf16),
            "wk": _pack_qk(W_K[hs], b_K[hs]).astype(bf16),
            "wv": _pack_v(W_V[hs], b_V[hs]).astype(bf16),
            "wo": np.ascontiguousarray(W_O[hs].reshape(256, M)).astype(bf16),
            "mk": mk.astype(bf16),
            "e2": e2.astype(bf16),
            "cst": cstv.astype(bf16),
        })
    return in_maps


def gather(results, b_O):
    out = np.zeros((2, S, M), np.float32)
    for c in range(N_CORES):
        out[c // 4] += np.asarray(results[c]["out"], dtype=np.float32)
    out += np.asarray(b_O, np.float32)[None, None, :]
    return out


def _run(in_maps, trace=False, zero_bias=False, **kw):
    nc = _get_program(zero_bias)
    return bass_utils.run_bass_kernel_spmd(
        nc, in_maps, core_ids=list(range(N_CORES)), trace=trace, **kw)


def all_zero_bias(b_Q, b_K, b_V):
    return (not np.any(np.asarray(b_Q)) and not np.any(np.asarray(b_K))
            and not np.any(np.asarray(b_V)))


def kernel(normalized_resid_pre, W_Q, b_Q, W_K, b_K, W_V, b_V, W_O, b_O):
    in_maps = prepare_in_maps(normalized_resid_pre, W_Q, b_Q, W_K, b_K, W_V,
                              b_V, W_O, b_O)
    res = _run(in_maps, zero_bias=all_zero_bias(b_Q, b_K, b_V))
    return gather(res.results, b_O)


# revision 16
# speedup vs baseline: 1.0249x; 1.0249x over previous
"""Trainium2 Bass kernel for nn_Attention_46780783788294.

Multi-head causal-ish attention (mask fills with 0.0, not -inf) for
x:[2,2048,1024], 16 heads of d_head=64, fp32 in/out, bf16 compute.

Sharding: 8 cores = 2 batches x 4 head-groups (4 heads each). Each core
computes its batch/head-group partial output [2048,1024] (bf16); host sums
the 4 partials per batch in fp32 and adds b_O.

Per-core device program (all-transposed "S^T" layout, bf16 matmuls with
fp32 PSUM accumulation):
  xT_aug [1025,2048] (x^T plus ones row) and packed/augmented weights come
  from the host. QT/KT computed per head-pair [128,2048] (d on partitions);
  V computed in natural [k,d] layout [128,260] per k-block with a per-head
  ones column (from the bias-row trick) so the AV matmul accumulates the
  softmax denominator for free. Scores are built k-on-partitions so
  P = exp(mask * s / 8) feeds the AV matmul directly with no transposes;
  masked entries give exp(0)=1 exactly as the reference's 0.0-fill softmax
  requires. Fully-masked k-blocks are skipped: their contribution (suffix
  column-sums of V) is added analytically via ones-rhs matmuls. The two
  heads of a pair share one [128,1024] scores psum so each mask-mul/exp
  covers both heads in one instruction. Chunk-outer loop: each q-chunk is
  attended, divided, output-projected and DMA'd out before the next, so
  PE/ACT/DVE/DMA stay overlapped throughout.
"""

import os
import sys

import numpy as np


def _ensure_concourse():
    try:
        import concourse  # noqa: F401
    except ImportError:
        for p in ("/root/.axon_site", "/root/.axon_site/_ro/trn_rl_repo",
                  "/root/.axon_site/_ro/pypackages", "/opt/trn_rl_repo"):
            if os.path.isdir(p) and p not in sys.path:
                sys.path.append(p)


_ensure_concourse()

import concourse.bass as bass  # noqa: E402
import concourse.tile as tile  # noqa: E402
from concourse import bacc, mybir  # noqa: E402
from concourse import bass_utils  # noqa: E402
from contextlib import ExitStack  # noqa: E402

F32 = mybir.dt.float32
BF16 = mybir.dt.bfloat16
EXP = mybir.ActivationFunctionType.Exp

S = 2048      # sequence length
M = 1024      # d_model
DH = 64       # d_head
HL = 4        # heads per core
NP = 2        # head pairs per core
CH = 512      # q-chunk width
NCH = S // CH     # 4 q chunks
KB = S // 128     # 16 k blocks
MB = M // 128     # 8 m blocks
N_CORES = 8


def _emit(tc, nc, d, zero_bias):
    mm = nc.tensor.matmul
    with ExitStack() as ctx:
        # ---- persistent pools ----
        qkp = ctx.enter_context(tc.tile_pool(name="qkp", bufs=1))
        vp = ctx.enter_context(tc.tile_pool(name="vp", bufs=1))
        wop = ctx.enter_context(tc.tile_pool(name="wop", bufs=1))
        cst = ctx.enter_context(tc.tile_pool(name="cst", bufs=1))
        dnp = ctx.enter_context(tc.tile_pool(name="dnp", bufs=1))
        z2p = ctx.enter_context(tc.tile_pool(name="z2p", bufs=1))
        xp = ctx.enter_context(tc.tile_pool(name="xp", bufs=1))
        wp = ctx.enter_context(tc.tile_pool(name="wp", bufs=1))
        pp = ctx.enter_context(tc.tile_pool(name="pp", bufs=3))
        stg = ctx.enter_context(tc.tile_pool(name="stg", bufs=2))
        op_sb = ctx.enter_context(tc.tile_pool(name="op_sb", bufs=3))
        # PSUM budget is 8 banks, statically split: sps 2x2, zps 2x1, and a
        # single 2-slot pool shared by every 1-bank psum use
        psX = ctx.enter_context(tc.tile_pool(name="psX", bufs=2, space="PSUM"))
        psS = ctx.enter_context(tc.tile_pool(name="psS", bufs=2, space="PSUM"))
        psZ = ctx.enter_context(tc.tile_pool(name="psZ", bufs=1, space="PSUM"))

        qt = [qkp.tile([128, S], BF16, name=f"qt{p}") for p in range(NP)]
        kt = [qkp.tile([128, S], BF16, name=f"kt{p}") for p in range(NP)]
        vt = [vp.tile([128, 260], BF16, name=f"vt{j}") for j in range(KB)]
        wo_t = [wop.tile([128, M], BF16, name=f"wo{p}") for p in range(NP)]
        dtri = cst.tile([128, 256], BF16, name="dtri")
        e2_t = cst.tile([2, 128], BF16, name="e2")
        ones_row = cst.tile([1, CH], BF16, name="ones_row")
        ones_col = cst.tile([128, 1], BF16, name="ones_col")
        dnc = [[dnp.tile([2, CH], F32, name=f"dnc{c}_{p}")
                for p in range(NP)] for c in range(NCH)]
        z2u = [z2p.tile([128, S], BF16, name=f"z2u{p}") for p in range(NP)]
        sfx = [dnp.tile([1, 260], BF16, name=f"sfx{c}") for c in range(3)]
        b_sb = {j: dnp.tile([1, 260], BF16, name=f"bsb{j}") for j in range(KB)
                if j % 4}

        xt_t = [xp.tile([128, S], BF16, name=f"xt{mb}") for mb in range(MB)]
        wq_t = [wp.tile([128, 256], BF16, name=f"wq{mb}") for mb in range(MB)]
        wk_t = [wp.tile([128, 256], BF16, name=f"wk{mb}") for mb in range(MB)]
        wv_t = [wp.tile([128, 260], BF16, name=f"wv{mb}") for mb in range(MB)]
        if not zero_bias:
            xt_ones = xp.tile([1, S], BF16, name="xt_ones")
            wq_b = wp.tile([1, 256], BF16, name="wq_b")
            wk_b = wp.tile([1, 256], BF16, name="wk_b")
            wv_b = wp.tile([1, 260], BF16, name="wv_b")

        # DMA order: what attention chunk 0 needs first (wq/wk, x^T chunk 0,
        # wv, masks), then the rest of x^T; W_O last. Intro loads are split
        # across both HWDGE dispatch engines (sync + scalar) since the
        # scalar engine is idle until the first exp.
        for mb in range(MB):
            ea = nc.scalar if mb % 2 else nc.sync
            eb = nc.sync if mb % 2 else nc.scalar
            ea.dma_start(wq_t[mb][:], d["wq"][128 * mb:128 * (mb + 1), :])
            eb.dma_start(xt_t[mb][:, 0:CH],
                         d["xt"][128 * mb:128 * (mb + 1), 0:CH])
            ea.dma_start(wk_t[mb][:], d["wk"][128 * mb:128 * (mb + 1), :])
        for mb in range(MB):
            eng = nc.sync if mb % 2 else nc.scalar
            eng.dma_start(wv_t[mb][:], d["wv"][128 * mb:128 * (mb + 1), :])
        if not zero_bias:
            nc.sync.dma_start(wq_b[:], d["wq"][1024:1025, :])
            nc.sync.dma_start(wk_b[:], d["wk"][1024:1025, :])
            nc.sync.dma_start(wv_b[:], d["wv"][1024:1025, :])
            nc.sync.dma_start(xt_ones[:], d["xt"][1024:1025, :])
        nc.scalar.dma_start(dtri[:], d["mk"][:])
        nc.sync.dma_start(ones_row[:], d["cst"][0:1, :])
        nc.sync.dma_start(ones_col[:], d["cst"][0:128, 0:1])
        nc.sync.dma_start(e2_t[:], d["e2"][:])
        for c in range(1, NCH):
            for mb in range(MB):
                nc.sync.dma_start(
                    xt_t[mb][:, CH * c:CH * (c + 1)],
                    d["xt"][128 * mb:128 * (mb + 1), CH * c:CH * (c + 1)])
        for p in range(NP):
            nc.sync.dma_start(wo_t[p][:], d["wo"][128 * p:128 * (p + 1), :])

        def emit_v(j):
            ps = psX.tile([128, 260], F32, name="psv", tag="px")
            for mb in range(MB):
                mm(ps[:], xt_t[mb][:, 128 * j:128 * (j + 1)],
                   wv_t[mb][:], start=(mb == 0),
                   stop=(zero_bias and mb == MB - 1))
            if not zero_bias:
                mm(ps[:], xt_ones[:, 128 * j:128 * (j + 1)], wv_b[:],
                   start=False, stop=True)
            nc.vector.tensor_copy(vt[j][:], ps[:])
            if zero_bias:
                # ones columns (the bias-row trick needs them even with zero
                # biases): set cols 64,129,194,259 to 1.0
                oc = vt[j].rearrange("p (h c) -> p h c", c=65)[:, :, 64]
                nc.gpsimd.memset(oc, 1.0)
            if j % 4:
                # per-block column sums of V_aug, for the fully-masked
                # column prefix of diagonal blocks
                bs = psX.tile([1, 260], F32, name="psb", tag="px")
                mm(bs[:], ones_col[:], vt[j][:], start=True, stop=True)
                nc.vector.tensor_copy(b_sb[j][:], bs[:])

        def emit_qk(p, which, c):
            dst, wt = (qt, wq_t) if which == 0 else (kt, wk_t)
            ps = psX.tile([128, CH], F32, name="psqk", tag="px")
            for mb in range(MB):
                mm(ps[:], wt[mb][:, 128 * p:128 * (p + 1)],
                   xt_t[mb][:, CH * c:CH * (c + 1)],
                   start=(mb == 0), stop=(zero_bias and mb == MB - 1))
            if not zero_bias:
                wb = wq_b if which == 0 else wk_b
                mm(ps[:], wb[:, 128 * p:128 * (p + 1)],
                   xt_ones[:, CH * c:CH * (c + 1)], start=False, stop=True)
            nc.vector.tensor_copy(dst[p][:, CH * c:CH * (c + 1)], ps[:])

        zps_by_chunk = {}

        def emit_attn_head(ch):
            # scores/mask/exp/AV for all k-blocks of chunk ch (both pairs)
            nj = 4 * ch + 4
            zl = []
            for p in range(NP):
                h0, h1 = 2 * p, 2 * p + 1
                zps = [psZ.tile([65, CH], F32, name=f"zps{half}",
                                tag=f"zps{half}") for half in range(2)]
                zl.append(zps)
                for j in range(nj):
                    # both heads of the pair share one [128,1024] scores
                    # psum: one mask-mul + one exp per j. For diagonal
                    # blocks only the unmasked column suffix [w0:512) is
                    # computed; the fully-masked prefix contributes
                    # exp(0)=1 per element, added analytically from the
                    # block's V column sums.
                    r = j - 4 * ch
                    w0 = 128 * r if r > 0 else 0
                    n = CH - w0
                    last = ch == 3 and j == nj - 1
                    sps = psS.tile([128, 2 * CH], F32, name="sps", tag="sps")
                    mm(sps[:, w0:CH],
                       kt[p][0:64, 128 * j:128 * (j + 1)],
                       qt[p][0:64, CH * ch + w0:CH * (ch + 1)],
                       start=True, stop=True)
                    mm(sps[:, CH + w0:2 * CH],
                       kt[p][64:128, 128 * j:128 * (j + 1)],
                       qt[p][64:128, CH * ch + w0:CH * (ch + 1)],
                       start=True, stop=True)
                    sps3 = sps.rearrange("p (t c) -> p t c", t=2)
                    pt = pp.tile([128, 2 * CH], BF16, name="pt", tag="pt")
                    pt3 = pt.rearrange("p (t c) -> p t c", t=2)
                    if r >= 0:
                        # triangular mask on the 128-wide diagonal strip of
                        # both heads at once
                        strip = sps3[:, :, w0:w0 + 128]
                        dtri3 = dtri.rearrange("p (t c) -> p t c", t=2)
                        nc.vector.tensor_mul(strip, strip, dtri3)
                    if w0:
                        nc.scalar.activation(pt3[:, :, w0:CH],
                                             sps3[:, :, w0:CH], EXP,
                                             scale=0.125)
                    else:
                        nc.scalar.activation(pt[:], sps[:], EXP, scale=0.125)
                    mm(zps[0][:, w0:CH], vt[j][:, 65 * h0:65 * h0 + 65],
                       pt[:, w0:CH], start=(j == 0),
                       stop=(last and not w0))
                    mm(zps[1][:, w0:CH], vt[j][:, 65 * h1:65 * h1 + 65],
                       pt[:, CH + w0:2 * CH], start=(j == 0),
                       stop=(last and not w0))
                    if w0:
                        mm(zps[0][:, 0:w0], b_sb[j][:, 65 * h0:65 * h0 + 65],
                           ones_row[:, 0:w0], start=False, stop=last)
                        mm(zps[1][:, 0:w0], b_sb[j][:, 65 * h1:65 * h1 + 65],
                           ones_row[:, 0:w0], start=False, stop=last)
            zps_by_chunk[ch] = zl

        def emit_attn_zrel(ch):
            # suffix contribution + denominator/z extraction for chunk ch —
            # releases the z psum slots as early as possible
            zl = zps_by_chunk.pop(ch)
            for p in range(NP):
                for half in range(2):
                    h = 2 * p + half
                    hb = 64 * half
                    if ch < 3:
                        mm(zl[p][half][:], sfx[ch][:, 65 * h:65 * h + 65],
                           ones_row[:], start=False, stop=True)
                    # stage denom row at partition 0, DMA-scatter into
                    # dnc[ch] (compute-engine APs need 32-aligned base
                    # partitions; DMA APs don't)
                    dnst = pp.tile([1, CH], F32, name="dnst", tag="dnst",
                                   bufs=4)
                    nc.vector.tensor_copy(dnst[:], zl[p][half][64:65, :])
                    nc.sync.dma_start(dnc[ch][p][half:half + 1, :], dnst[:])
                    nc.vector.tensor_copy(
                        z2u[p][hb:hb + 64, CH * ch:CH * (ch + 1)],
                        zl[p][half][0:64, :])

        def emit_divE(ch):
            # divide chunk ch's z by the softmax denominators, project to
            # the output and stream to DRAM
            for p in range(NP):
                rdc = stg.tile([2, CH], F32, name="rdc", tag="rdc")
                nc.vector.reciprocal_approx_fast(rdc[:], dnc[ch][p][:])
                rdcb = stg.tile([2, CH], BF16, name="rdcb", tag="rdcb")
                nc.vector.tensor_copy(rdcb[:], rdc[:])
                bc = psX.tile([128, CH], F32, name="bc", tag="px")
                mm(bc[:], e2_t[:], rdcb[:], start=True, stop=True)
                nc.vector.tensor_mul(
                    z2u[p][:, CH * ch:CH * (ch + 1)],
                    z2u[p][:, CH * ch:CH * (ch + 1)], bc[:])

            for q in range(4 * ch, 4 * ch + 4):
                for mc in range(2):
                    ops = psX.tile([128, CH], F32, name="ops", tag="px")
                    for p in range(NP):
                        mm(ops[:], z2u[p][:, 128 * q:128 * (q + 1)],
                           wo_t[p][:, CH * mc:CH * (mc + 1)],
                           start=(p == 0), stop=(p == 1))
                    osb = op_sb.tile([128, CH], BF16, name="osb", tag="osb")
                    nc.vector.tensor_copy(osb[:], ops[:])
                    nc.sync.dma_start(
                        d["out"][128 * q:128 * (q + 1), CH * mc:CH * (mc + 1)],
                        osb[:])

        # ---- emission: interleave projections, attention, division and
        # output so every engine has work throughout. Chunk c's division +
        # output projection is emitted after chunk c+1's attention head so
        # the PE never blocks on the (latency-heavy) division chain. ----
        for p in range(NP):
            emit_qk(p, 0, 0)
            emit_qk(p, 1, 0)
        for j in range(4):
            emit_v(j)
        emit_attn_head(0)
        for j in range(4, KB):
            emit_v(j)
        for p in range(NP):
            for c in range(1, NCH):
                emit_qk(p, 0, c)
                emit_qk(p, 1, c)
        # suffix column-sums of V_aug for fully-masked regions
        for c in range(3):
            ps = psX.tile([1, 260], F32, name="pssfx", tag="px")
            for j in range(4 * c + 4, KB):
                mm(ps[:], ones_col[:], vt[j][:],
                   start=(j == 4 * c + 4), stop=(j == KB - 1))
            nc.vector.tensor_copy(sfx[c][:], ps[:])
        emit_attn_zrel(0)
        for ch in range(1, NCH):
            emit_attn_head(ch)
            emit_divE(ch - 1)
            emit_attn_zrel(ch)
        emit_divE(3)


def build_program(zero_bias=False):
    nc = bacc.Bacc("TRN2", target_bir_lowering=False, debug=False,
                   num_devices=N_CORES)
    d = {
        "xt": nc.dram_tensor("xt", [1025, S], BF16, kind="ExternalInput").ap(),
        "wq": nc.dram_tensor("wq", [1025, 256], BF16, kind="ExternalInput").ap(),
        "wk": nc.dram_tensor("wk", [1025, 256], BF16, kind="ExternalInput").ap(),
        "wv": nc.dram_tensor("wv", [1025, 260], BF16, kind="ExternalInput").ap(),
        "wo": nc.dram_tensor("wo", [256, M], BF16, kind="ExternalInput").ap(),
        "mk": nc.dram_tensor("mk", [128, 256], BF16, kind="ExternalInput").ap(),
        "e2": nc.dram_tensor("e2", [2, 128], BF16, kind="ExternalInput").ap(),
        "cst": nc.dram_tensor("cst", [128, CH], BF16, kind="ExternalInput").ap(),
        "out": nc.dram_tensor("out", [S, M], BF16, kind="ExternalOutput").ap(),
    }
    with tile.TileContext(nc) as tc:
        _emit(tc, nc, d, zero_bias)
    nc.compile()
    return nc


_CACHE = {}


def _get_program(zero_bias=False):
    key = ("nc", zero_bias)
    if key not in _CACHE:
        _CACHE[key] = build_program(zero_bias)
    return _CACHE[key]


def _pack_qk(w4, b4):
    # w4 [4,1024,64], b4 [4,64] -> [1025, 256] (m-major, head-major cols)
    r = np.empty((1025, 256), np.float32)
    r[:1024] = w4.transpose(1, 0, 2).reshape(1024, 256)
    r[1024] = b4.reshape(256)
    return r


def _pack_v(w4, b4):
    # [1025, 260]: per head 64 W_V cols + a ones-generating column
    r = np.zeros((1025, 260), np.float32)
    for h in range(4):
        r[:1024, 65 * h:65 * h + 64] = w4[h]
        r[1024, 65 * h:65 * h + 64] = b4[h]
        r[1024, 65 * h + 64] = 1.0
    return r


def prepare_in_maps(normalized_resid_pre, W_Q, b_Q, W_K, b_K, W_V, b_V, W_O,
                    b_O):
    import ml_dtypes
    bf16 = ml_dtypes.bfloat16
    x = np.asarray(normalized_resid_pre, np.float32)
    W_Q = np.asarray(W_Q, np.float32)
    b_Q = np.asarray(b_Q, np.float32)
    W_K = np.asarray(W_K, np.float32)
    b_K = np.asarray(b_K, np.float32)
    W_V = np.asarray(W_V, np.float32)
    b_V = np.asarray(b_V, np.float32)
    W_O = np.asarray(W_O, np.float32)

    tri = np.triu(np.ones((128, 128), np.float32))  # [k,q]: 1 where k <= q
    mk = np.tile(tri, (1, 2))  # both heads of a pair side by side
    e2 = np.zeros((2, 128), np.float32)
    e2[0, :64] = 1.0
    e2[1, 64:] = 1.0
    cstv = np.ones((128, CH), np.float32)

    xts = []
    for b in range(2):
        xt = np.empty((1025, S), np.float32)
        xt[:1024] = x[b].T
        xt[1024] = 1.0
        xts.append(xt.astype(bf16))

    in_maps = []
    for c in range(N_CORES):
        b, g = divmod(c, 4)
        hs = slice(4 * g, 4 * g + 4)
        in_maps.append({
            "xt": xts[b],
            "wq": _pack_qk(W_Q[hs], b_Q[hs]).astype(bf16),
            "wk": _pack_qk(W_K[hs], b_K[hs]).astype(bf16),
            "wv": _pack_v(W_V[hs], b_V[hs]).astype(bf16),
            "wo": np.ascontiguousarray(W_O[hs].reshape(256, M)).astype(bf16),
            "mk": mk.astype(bf16),
            "e2": e2.astype(bf16),
            "cst": cstv.astype(bf16),
        })
    return in_maps


def gather(results, b_O):
    out = np.zeros((2, S, M), np.float32)
    for c in range(N_CORES):
        out[c // 4] += np.asarray(results[c]["out"], dtype=np.float32)
    out += np.asarray(b_O, np.float32)[None, None, :]
    return out


def _run(in_maps, trace=False, zero_bias=False, **kw):
    nc = _get_program(zero_bias)
    return bass_utils.run_bass_kernel_spmd(
        nc, in_maps, core_ids=list(range(N_CORES)), trace=trace, **kw)


def all_zero_bias(b_Q, b_K, b_V):
    return (not np.any(np.asarray(b_Q)) and not np.any(np.asarray(b_K))
            and not np.any(np.asarray(b_V)))


def kernel(normalized_resid_pre, W_Q, b_Q, W_K, b_K, W_V, b_V, W_O, b_O):
    in_maps = prepare_in_maps(normalized_resid_pre, W_Q, b_Q, W_K, b_K, W_V,
                              b_V, W_O, b_O)
    res = _run(in_maps, zero_bias=all_zero_bias(b_Q, b_K, b_V))
    return gather(res.results, b_O)


# revision 17
# speedup vs baseline: 1.1531x; 1.1251x over previous
"""Trainium2 Bass kernel for nn_Attention_46780783788294.

Multi-head causal-ish attention (mask fills with 0.0, not -inf) for
x:[2,2048,1024], 16 heads of d_head=64, fp32 in/out, bf16 compute.

Sharding: 8 cores = 2 batches x 4 head-groups (4 heads each). Each core
computes its batch/head-group partial output [2048,1024] (bf16); host sums
the 4 partials per batch in fp32 and adds b_O.

Per-core device program (all-transposed "S^T" layout, bf16 matmuls with
fp32 PSUM accumulation):
  xT_aug [1025,2048] (x^T plus ones row) and packed/augmented weights come
  from the host. QT/KT computed per head-pair [128,2048] (d on partitions);
  V computed in natural [k,d] layout [128,260] per k-block with a per-head
  ones column (from the bias-row trick) so the AV matmul accumulates the
  softmax denominator for free. Scores are built k-on-partitions so
  P = exp(mask * s / 8) feeds the AV matmul directly with no transposes;
  masked entries give exp(0)=1 exactly as the reference's 0.0-fill softmax
  requires. Fully-masked k-blocks are skipped: their contribution (suffix
  column-sums of V) is added analytically via ones-rhs matmuls. The two
  heads of a pair share one [128,1024] scores psum so each mask-mul/exp
  covers both heads in one instruction. Chunk-outer loop: each q-chunk is
  attended, divided, output-projected and DMA'd out before the next, so
  PE/ACT/DVE/DMA stay overlapped throughout.
"""

import os
import sys

import numpy as np


def _ensure_concourse():
    try:
        import concourse  # noqa: F401
    except ImportError:
        for p in ("/root/.axon_site", "/root/.axon_site/_ro/trn_rl_repo",
                  "/root/.axon_site/_ro/pypackages", "/opt/trn_rl_repo"):
            if os.path.isdir(p) and p not in sys.path:
                sys.path.append(p)


_ensure_concourse()

import concourse.bass as bass  # noqa: E402
import concourse.tile as tile  # noqa: E402
from concourse import bacc, mybir  # noqa: E402
from concourse import bass_utils  # noqa: E402
from contextlib import ExitStack  # noqa: E402

F32 = mybir.dt.float32
BF16 = mybir.dt.bfloat16
EXP = mybir.ActivationFunctionType.Exp

S = 2048      # sequence length
M = 1024      # d_model
DH = 64       # d_head
HL = 4        # heads per core
NP = 2        # head pairs per core
CH = 512      # q-chunk width
NCH = S // CH     # 4 q chunks
KB = S // 128     # 16 k blocks
MB = M // 128     # 8 m blocks
N_CORES = 8


def _emit(tc, nc, d, zero_bias):
    mm = nc.tensor.matmul
    with ExitStack() as ctx:
        # ---- persistent pools ----
        qkp = ctx.enter_context(tc.tile_pool(name="qkp", bufs=1))
        vp = ctx.enter_context(tc.tile_pool(name="vp", bufs=1))
        wop = ctx.enter_context(tc.tile_pool(name="wop", bufs=1))
        cst = ctx.enter_context(tc.tile_pool(name="cst", bufs=1))
        dnp = ctx.enter_context(tc.tile_pool(name="dnp", bufs=1))
        z2p = ctx.enter_context(tc.tile_pool(name="z2p", bufs=1))
        xp = ctx.enter_context(tc.tile_pool(name="xp", bufs=1))
        wp = ctx.enter_context(tc.tile_pool(name="wp", bufs=1))
        pp = ctx.enter_context(tc.tile_pool(name="pp", bufs=3))
        stg = ctx.enter_context(tc.tile_pool(name="stg", bufs=2))
        op_sb = ctx.enter_context(tc.tile_pool(name="op_sb", bufs=3))
        # PSUM budget is 8 banks, statically split: sps 2x2, zps 2x1, and a
        # single 2-slot pool shared by every 1-bank psum use
        psX = ctx.enter_context(tc.tile_pool(name="psX", bufs=2, space="PSUM"))
        psS = ctx.enter_context(tc.tile_pool(name="psS", bufs=2, space="PSUM"))
        psZ = ctx.enter_context(tc.tile_pool(name="psZ", bufs=1, space="PSUM"))

        qt = [qkp.tile([128, S], BF16, name=f"qt{p}") for p in range(NP)]
        kt = [qkp.tile([128, S], BF16, name=f"kt{p}") for p in range(NP)]
        vt = [vp.tile([128, 260], BF16, name=f"vt{j}") for j in range(KB)]
        wo_t = [wop.tile([128, M], BF16, name=f"wo{p}") for p in range(NP)]
        dtri = cst.tile([128, 256], BF16, name="dtri")
        e2_t = cst.tile([2, 128], BF16, name="e2")
        ones_row = cst.tile([1, CH], BF16, name="ones_row")
        ones_col = cst.tile([128, 1], BF16, name="ones_col")
        dnc = [[dnp.tile([2, CH], F32, name=f"dnc{c}_{p}")
                for p in range(NP)] for c in range(NCH)]
        z2u = [z2p.tile([128, S], BF16, name=f"z2u{p}") for p in range(NP)]
        sfx = [dnp.tile([1, 260], BF16, name=f"sfx{c}") for c in range(3)]
        b_sb = {j: dnp.tile([1, 260], BF16, name=f"bsb{j}") for j in range(KB)
                if j % 4}

        xt_t = [xp.tile([128, S], BF16, name=f"xt{mb}") for mb in range(MB)]
        wq_t = [wp.tile([128, 256], BF16, name=f"wq{mb}") for mb in range(MB)]
        wk_t = [wp.tile([128, 256], BF16, name=f"wk{mb}") for mb in range(MB)]
        wv_t = [wp.tile([128, 260], BF16, name=f"wv{mb}") for mb in range(MB)]
        if not zero_bias:
            xt_ones = xp.tile([1, S], BF16, name="xt_ones")
            wq_b = wp.tile([1, 256], BF16, name="wq_b")
            wk_b = wp.tile([1, 256], BF16, name="wk_b")
            wv_b = wp.tile([1, 260], BF16, name="wv_b")

        # DMA order: what attention chunk 0 needs first (wq/wk, x^T chunk 0,
        # wv, masks), then the rest of x^T; W_O last. Intro loads are split
        # across both HWDGE dispatch engines (sync + scalar) since the
        # scalar engine is idle until the first exp.
        for mb in range(MB):
            ea = nc.scalar if mb % 2 else nc.sync
            eb = nc.sync if mb % 2 else nc.scalar
            ea.dma_start(wq_t[mb][:], d["wq"][128 * mb:128 * (mb + 1), :])
            eb.dma_start(xt_t[mb][:, 0:CH],
                         d["xt"][128 * mb:128 * (mb + 1), 0:CH])
            ea.dma_start(wk_t[mb][:], d["wk"][128 * mb:128 * (mb + 1), :])
        for mb in range(MB):
            eng = nc.sync if mb % 2 else nc.scalar
            eng.dma_start(wv_t[mb][:], d["wv"][128 * mb:128 * (mb + 1), :])
        if not zero_bias:
            nc.sync.dma_start(wq_b[:], d["wq"][1024:1025, :])
            nc.sync.dma_start(wk_b[:], d["wk"][1024:1025, :])
            nc.sync.dma_start(wv_b[:], d["wv"][1024:1025, :])
            nc.sync.dma_start(xt_ones[:], d["xt"][1024:1025, :])
        nc.scalar.dma_start(dtri[:], d["mk"][:])
        nc.sync.dma_start(ones_row[:], d["cst"][0:1, :])
        nc.sync.dma_start(ones_col[:], d["cst"][0:128, 0:1])
        nc.sync.dma_start(e2_t[:], d["e2"][:])
        for c in range(1, NCH):
            for mb in range(MB):
                nc.sync.dma_start(
                    xt_t[mb][:, CH * c:CH * (c + 1)],
                    d["xt"][128 * mb:128 * (mb + 1), CH * c:CH * (c + 1)])
        for p in range(NP):
            nc.sync.dma_start(wo_t[p][:], d["wo"][128 * p:128 * (p + 1), :])

        def emit_v(j):
            ps = psX.tile([128, 260], F32, name="psv", tag="px")
            for mb in range(MB):
                mm(ps[:], xt_t[mb][:, 128 * j:128 * (j + 1)],
                   wv_t[mb][:], start=(mb == 0),
                   stop=(zero_bias and mb == MB - 1))
            if not zero_bias:
                mm(ps[:], xt_ones[:, 128 * j:128 * (j + 1)], wv_b[:],
                   start=False, stop=True)
            nc.vector.tensor_copy(vt[j][:], ps[:])
            if zero_bias:
                # ones columns (the bias-row trick needs them even with zero
                # biases): set cols 64,129,194,259 to 1.0
                oc = vt[j].rearrange("p (h c) -> p h c", c=65)[:, :, 64]
                nc.gpsimd.memset(oc, 1.0)
            if j % 4:
                # per-block column sums of V_aug, for the fully-masked
                # column prefix of diagonal blocks
                bs = psX.tile([1, 260], F32, name="psb", tag="px")
                mm(bs[:], ones_col[:], vt[j][:], start=True, stop=True)
                nc.vector.tensor_copy(b_sb[j][:], bs[:])

        def emit_qk(p, which, c):
            dst, wt = (qt, wq_t) if which == 0 else (kt, wk_t)
            ps = psX.tile([128, CH], F32, name="psqk", tag="px")
            for mb in range(MB):
                mm(ps[:], wt[mb][:, 128 * p:128 * (p + 1)],
                   xt_t[mb][:, CH * c:CH * (c + 1)],
                   start=(mb == 0), stop=(zero_bias and mb == MB - 1))
            if not zero_bias:
                wb = wq_b if which == 0 else wk_b
                mm(ps[:], wb[:, 128 * p:128 * (p + 1)],
                   xt_ones[:, CH * c:CH * (c + 1)], start=False, stop=True)
            nc.vector.tensor_copy(dst[p][:, CH * c:CH * (c + 1)], ps[:])

        zps_by_chunk = {}

        def emit_attn_head(ch):
            # scores/mask/exp/AV for all k-blocks of chunk ch (both pairs)
            nj = 4 * ch + 4
            zl = []
            for p in range(NP):
                h0, h1 = 2 * p, 2 * p + 1
                zps = [psZ.tile([65, CH], F32, name=f"zps{half}",
                                tag=f"zps{half}") for half in range(2)]
                zl.append(zps)
                for j in range(nj):
                    # both heads of the pair share one [128,1024] scores
                    # psum: one mask-mul + one exp per j. For diagonal
                    # blocks only the unmasked column suffix [w0:512) is
                    # computed; the fully-masked prefix contributes
                    # exp(0)=1 per element, added analytically from the
                    # block's V column sums.
                    r = j - 4 * ch
                    w0 = 128 * r if r > 0 else 0
                    n = CH - w0
                    last = ch == 3 and j == nj - 1
                    sps = psS.tile([128, 2 * CH], F32, name="sps", tag="sps")
                    mm(sps[:, w0:CH],
                       kt[p][0:64, 128 * j:128 * (j + 1)],
                       qt[p][0:64, CH * ch + w0:CH * (ch + 1)],
                       start=True, stop=True)
                    mm(sps[:, CH + w0:2 * CH],
                       kt[p][64:128, 128 * j:128 * (j + 1)],
                       qt[p][64:128, CH * ch + w0:CH * (ch + 1)],
                       start=True, stop=True)
                    sps3 = sps.rearrange("p (t c) -> p t c", t=2)
                    pt = pp.tile([128, 2 * CH], BF16, name="pt", tag="pt")
                    pt3 = pt.rearrange("p (t c) -> p t c", t=2)
                    if r >= 0:
                        # triangular mask on the 128-wide diagonal strip of
                        # both heads at once
                        strip = sps3[:, :, w0:w0 + 128]
                        dtri3 = dtri.rearrange("p (t c) -> p t c", t=2)
                        nc.vector.tensor_mul(strip, strip, dtri3)
                    if w0:
                        nc.scalar.activation(pt3[:, :, w0:CH],
                                             sps3[:, :, w0:CH], EXP,
                                             scale=0.125)
                    else:
                        nc.scalar.activation(pt[:], sps[:], EXP, scale=0.125)
                    mm(zps[0][:, w0:CH], vt[j][:, 65 * h0:65 * h0 + 65],
                       pt[:, w0:CH], start=(j == 0),
                       stop=(last and not w0))
                    mm(zps[1][:, w0:CH], vt[j][:, 65 * h1:65 * h1 + 65],
                       pt[:, CH + w0:2 * CH], start=(j == 0),
                       stop=(last and not w0))
                    if w0:
                        mm(zps[0][:, 0:w0], b_sb[j][:, 65 * h0:65 * h0 + 65],
                           ones_row[:, 0:w0], start=False, stop=last)
                        mm(zps[1][:, 0:w0], b_sb[j][:, 65 * h1:65 * h1 + 65],
                           ones_row[:, 0:w0], start=False, stop=last)
            zps_by_chunk[ch] = zl

        def emit_attn_zrel(ch):
            # suffix contribution + denominator/z extraction for chunk ch —
            # releases the z psum slots as early as possible
            zl = zps_by_chunk.pop(ch)
            for p in range(NP):
                for half in range(2):
                    h = 2 * p + half
                    hb = 64 * half
                    if ch < 3:
                        mm(zl[p][half][:], sfx[ch][:, 65 * h:65 * h + 65],
                           ones_row[:], start=False, stop=True)
                    # stage denom row at partition 0, DMA-scatter into
                    # dnc[ch] (compute-engine APs need 32-aligned base
                    # partitions; DMA APs don't)
                    dnst = pp.tile([1, CH], F32, name="dnst", tag="dnst",
                                   bufs=4)
                    nc.vector.tensor_copy(dnst[:], zl[p][half][64:65, :])
                    nc.sync.dma_start(dnc[ch][p][half:half + 1, :], dnst[:])
                    nc.vector.tensor_copy(
                        z2u[p][hb:hb + 64, CH * ch:CH * (ch + 1)],
                        zl[p][half][0:64, :])

        def emit_divE(ch):
            # divide chunk ch's z by the softmax denominators, project to
            # the output and stream to DRAM
            for p in range(NP):
                rdc = stg.tile([2, CH], F32, name="rdc", tag="rdc")
                nc.vector.reciprocal_approx_fast(rdc[:], dnc[ch][p][:])
                rdcb = stg.tile([2, CH], BF16, name="rdcb", tag="rdcb")
                nc.vector.tensor_copy(rdcb[:], rdc[:])
                bc = psX.tile([128, CH], F32, name="bc", tag="px")
                mm(bc[:], e2_t[:], rdcb[:], start=True, stop=True)
                nc.vector.tensor_mul(
                    z2u[p][:, CH * ch:CH * (ch + 1)],
                    z2u[p][:, CH * ch:CH * (ch + 1)], bc[:])

            for q in range(4 * ch, 4 * ch + 4):
                for mc in range(2):
                    ops = psX.tile([128, CH], F32, name="ops", tag="px")
                    for p in range(NP):
                        mm(ops[:], z2u[p][:, 128 * q:128 * (q + 1)],
                           wo_t[p][:, CH * mc:CH * (mc + 1)],
                           start=(p == 0), stop=(p == 1))
                    osb = op_sb.tile([128, CH], BF16, name="osb", tag="osb")
                    nc.vector.tensor_copy(osb[:], ops[:])
                    nc.sync.dma_start(
                        d["out"][128 * q:128 * (q + 1), CH * mc:CH * (mc + 1)],
                        osb[:])

        # ---- emission: interleave projections, attention, division and
        # output so every engine has work throughout. Chunk c's division +
        # output projection is emitted after chunk c+1's attention head so
        # the PE never blocks on the (latency-heavy) division chain. ----
        for p in range(NP):
            emit_qk(p, 0, 0)
            emit_qk(p, 1, 0)
        for j in range(4):
            emit_v(j)
        emit_attn_head(0)
        for j in range(4, KB):
            emit_v(j)
        # suffix column-sums of V_aug for fully-masked regions
        for c in range(3):
            ps = psX.tile([1, 260], F32, name="pssfx", tag="px")
            for j in range(4 * c + 4, KB):
                mm(ps[:], ones_col[:], vt[j][:],
                   start=(j == 4 * c + 4), stop=(j == KB - 1))
            nc.vector.tensor_copy(sfx[c][:], ps[:])
        emit_attn_zrel(0)
        for ch in range(1, NCH):
            for p in range(NP):
                emit_qk(p, 0, ch)
                emit_qk(p, 1, ch)
            emit_attn_head(ch)
            emit_divE(ch - 1)
            emit_attn_zrel(ch)
        emit_divE(3)


def build_program(zero_bias=False):
    nc = bacc.Bacc("TRN2", target_bir_lowering=False, debug=False,
                   num_devices=N_CORES)
    d = {
        "xt": nc.dram_tensor("xt", [1025, S], BF16, kind="ExternalInput").ap(),
        "wq": nc.dram_tensor("wq", [1025, 256], BF16, kind="ExternalInput").ap(),
        "wk": nc.dram_tensor("wk", [1025, 256], BF16, kind="ExternalInput").ap(),
        "wv": nc.dram_tensor("wv", [1025, 260], BF16, kind="ExternalInput").ap(),
        "wo": nc.dram_tensor("wo", [256, M], BF16, kind="ExternalInput").ap(),
        "mk": nc.dram_tensor("mk", [128, 256], BF16, kind="ExternalInput").ap(),
        "e2": nc.dram_tensor("e2", [2, 128], BF16, kind="ExternalInput").ap(),
        "cst": nc.dram_tensor("cst", [128, CH], BF16, kind="ExternalInput").ap(),
        "out": nc.dram_tensor("out", [S, M], BF16, kind="ExternalOutput").ap(),
    }
    with tile.TileContext(nc) as tc:
        _emit(tc, nc, d, zero_bias)
    nc.compile()
    return nc


_CACHE = {}


def _get_program(zero_bias=False):
    key = ("nc", zero_bias)
    if key not in _CACHE:
        _CACHE[key] = build_program(zero_bias)
    return _CACHE[key]


def _pack_qk(w4, b4):
    # w4 [4,1024,64], b4 [4,64] -> [1025, 256] (m-major, head-major cols)
    r = np.empty((1025, 256), np.float32)
    r[:1024] = w4.transpose(1, 0, 2).reshape(1024, 256)
    r[1024] = b4.reshape(256)
    return r


def _pack_v(w4, b4):
    # [1025, 260]: per head 64 W_V cols + a ones-generating column
    r = np.zeros((1025, 260), np.float32)
    for h in range(4):
        r[:1024, 65 * h:65 * h + 64] = w4[h]
        r[1024, 65 * h:65 * h + 64] = b4[h]
        r[1024, 65 * h + 64] = 1.0
    return r


def prepare_in_maps(normalized_resid_pre, W_Q, b_Q, W_K, b_K, W_V, b_V, W_O,
                    b_O):
    import ml_dtypes
    bf16 = ml_dtypes.bfloat16
    x = np.asarray(normalized_resid_pre, np.float32)
    W_Q = np.asarray(W_Q, np.float32)
    b_Q = np.asarray(b_Q, np.float32)
    W_K = np.asarray(W_K, np.float32)
    b_K = np.asarray(b_K, np.float32)
    W_V = np.asarray(W_V, np.float32)
    b_V = np.asarray(b_V, np.float32)
    W_O = np.asarray(W_O, np.float32)

    tri = np.triu(np.ones((128, 128), np.float32))  # [k,q]: 1 where k <= q
    mk = np.tile(tri, (1, 2))  # both heads of a pair side by side
    e2 = np.zeros((2, 128), np.float32)
    e2[0, :64] = 1.0
    e2[1, 64:] = 1.0
    cstv = np.ones((128, CH), np.float32)

    xts = []
    for b in range(2):
        xt = np.empty((1025, S), np.float32)
        xt[:1024] = x[b].T
        xt[1024] = 1.0
        xts.append(xt.astype(bf16))

    in_maps = []
    for c in range(N_CORES):
        b, g = divmod(c, 4)
        hs = slice(4 * g, 4 * g + 4)
        in_maps.append({
            "xt": xts[b],
            "wq": _pack_qk(W_Q[hs], b_Q[hs]).astype(bf16),
            "wk": _pack_qk(W_K[hs], b_K[hs]).astype(bf16),
            "wv": _pack_v(W_V[hs], b_V[hs]).astype(bf16),
            "wo": np.ascontiguousarray(W_O[hs].reshape(256, M)).astype(bf16),
            "mk": mk.astype(bf16),
            "e2": e2.astype(bf16),
            "cst": cstv.astype(bf16),
        })
    return in_maps


def gather(results, b_O):
    out = np.zeros((2, S, M), np.float32)
    for c in range(N_CORES):
        out[c // 4] += np.asarray(results[c]["out"], dtype=np.float32)
    out += np.asarray(b_O, np.float32)[None, None, :]
    return out


def _run(in_maps, trace=False, zero_bias=False, **kw):
    nc = _get_program(zero_bias)
    return bass_utils.run_bass_kernel_spmd(
        nc, in_maps, core_ids=list(range(N_CORES)), trace=trace, **kw)


def all_zero_bias(b_Q, b_K, b_V):
    return (not np.any(np.asarray(b_Q)) and not np.any(np.asarray(b_K))
            and not np.any(np.asarray(b_V)))


def kernel(normalized_resid_pre, W_Q, b_Q, W_K, b_K, W_V, b_V, W_O, b_O):
    in_maps = prepare_in_maps(normalized_resid_pre, W_Q, b_Q, W_K, b_K, W_V,
                              b_V, W_O, b_O)
    res = _run(in_maps, zero_bias=all_zero_bias(b_Q, b_K, b_V))
    return gather(res.results, b_O)


# revision 18
# speedup vs baseline: 1.1749x; 1.0189x over previous
"""Trainium2 Bass kernel for nn_Attention_46780783788294.

Multi-head causal-ish attention (mask fills with 0.0, not -inf) for
x:[2,2048,1024], 16 heads of d_head=64, fp32 in/out, bf16 compute.

Sharding: 8 cores = 2 batches x 4 head-groups (4 heads each). Each core
computes its batch/head-group partial output [2048,1024] (bf16); host sums
the 4 partials per batch in fp32 and adds b_O.

Per-core device program (all-transposed "S^T" layout, bf16 matmuls with
fp32 PSUM accumulation):
  xT_aug [1025,2048] (x^T plus ones row) and packed/augmented weights come
  from the host. QT/KT computed per head-pair [128,2048] (d on partitions);
  V computed in natural [k,d] layout [128,260] per k-block with a per-head
  ones column (from the bias-row trick) so the AV matmul accumulates the
  softmax denominator for free. Scores are built k-on-partitions so
  P = exp(mask * s / 8) feeds the AV matmul directly with no transposes;
  masked entries give exp(0)=1 exactly as the reference's 0.0-fill softmax
  requires. Fully-masked k-blocks are skipped: their contribution (suffix
  column-sums of V) is added analytically via ones-rhs matmuls. The two
  heads of a pair share one [128,1024] scores psum so each mask-mul/exp
  covers both heads in one instruction. Chunk-outer loop: each q-chunk is
  attended, divided, output-projected and DMA'd out before the next, so
  PE/ACT/DVE/DMA stay overlapped throughout.
"""

import os
import sys

import numpy as np


def _ensure_concourse():
    try:
        import concourse  # noqa: F401
    except ImportError:
        for p in ("/root/.axon_site", "/root/.axon_site/_ro/trn_rl_repo",
                  "/root/.axon_site/_ro/pypackages", "/opt/trn_rl_repo"):
            if os.path.isdir(p) and p not in sys.path:
                sys.path.append(p)


_ensure_concourse()

import concourse.bass as bass  # noqa: E402
import concourse.tile as tile  # noqa: E402
from concourse import bacc, mybir  # noqa: E402
from concourse import bass_utils  # noqa: E402
from contextlib import ExitStack  # noqa: E402

F32 = mybir.dt.float32
BF16 = mybir.dt.bfloat16
EXP = mybir.ActivationFunctionType.Exp

S = 2048      # sequence length
M = 1024      # d_model
DH = 64       # d_head
HL = 4        # heads per core
NP = 2        # head pairs per core
CH = 512      # q-chunk width
NCH = S // CH     # 4 q chunks
KB = S // 128     # 16 k blocks
MB = M // 128     # 8 m blocks
N_CORES = 8


def _emit(tc, nc, d, zero_bias):
    mm = nc.tensor.matmul
    with ExitStack() as ctx:
        # ---- persistent pools ----
        qkp = ctx.enter_context(tc.tile_pool(name="qkp", bufs=1))
        vp = ctx.enter_context(tc.tile_pool(name="vp", bufs=1))
        wop = ctx.enter_context(tc.tile_pool(name="wop", bufs=1))
        cst = ctx.enter_context(tc.tile_pool(name="cst", bufs=1))
        dnp = ctx.enter_context(tc.tile_pool(name="dnp", bufs=1))
        z2p = ctx.enter_context(tc.tile_pool(name="z2p", bufs=1))
        xp = ctx.enter_context(tc.tile_pool(name="xp", bufs=1))
        wp = ctx.enter_context(tc.tile_pool(name="wp", bufs=1))
        pp = ctx.enter_context(tc.tile_pool(name="pp", bufs=4))
        stg = ctx.enter_context(tc.tile_pool(name="stg", bufs=3))
        op_sb = ctx.enter_context(tc.tile_pool(name="op_sb", bufs=4))
        # PSUM budget is 8 banks, statically split: sps 2x2, zps 2x1, and a
        # single 2-slot pool shared by every 1-bank psum use
        psX = ctx.enter_context(tc.tile_pool(name="psX", bufs=2, space="PSUM"))
        psS = ctx.enter_context(tc.tile_pool(name="psS", bufs=2, space="PSUM"))
        psZ = ctx.enter_context(tc.tile_pool(name="psZ", bufs=1, space="PSUM"))

        qt = [qkp.tile([128, S], BF16, name=f"qt{p}") for p in range(NP)]
        kt = [qkp.tile([128, S], BF16, name=f"kt{p}") for p in range(NP)]
        vt = [vp.tile([128, 260], BF16, name=f"vt{j}") for j in range(KB)]
        wo_t = [wop.tile([128, M], BF16, name=f"wo{p}") for p in range(NP)]
        dtri = cst.tile([128, 256], BF16, name="dtri")
        e2_t = cst.tile([2, 128], BF16, name="e2")
        ones_row = cst.tile([1, CH], BF16, name="ones_row")
        ones_col = cst.tile([128, 1], BF16, name="ones_col")
        dnc = [[dnp.tile([2, CH], F32, name=f"dnc{c}_{p}")
                for p in range(NP)] for c in range(NCH)]
        z2u = [z2p.tile([128, S], BF16, name=f"z2u{p}") for p in range(NP)]
        sfx = [dnp.tile([1, 260], BF16, name=f"sfx{c}") for c in range(3)]
        b_sb = {j: dnp.tile([1, 260], BF16, name=f"bsb{j}") for j in range(KB)
                if j % 4}

        xt_t = [xp.tile([128, S], BF16, name=f"xt{mb}") for mb in range(MB)]
        wq_t = [wp.tile([128, 256], BF16, name=f"wq{mb}") for mb in range(MB)]
        wk_t = [wp.tile([128, 256], BF16, name=f"wk{mb}") for mb in range(MB)]
        wv_t = [wp.tile([128, 260], BF16, name=f"wv{mb}") for mb in range(MB)]
        if not zero_bias:
            xt_ones = xp.tile([1, S], BF16, name="xt_ones")
            wq_b = wp.tile([1, 256], BF16, name="wq_b")
            wk_b = wp.tile([1, 256], BF16, name="wk_b")
            wv_b = wp.tile([1, 260], BF16, name="wv_b")

        # DMA order: what attention chunk 0 needs first (wq/wk, x^T chunk 0,
        # wv, masks), then the rest of x^T; W_O last. Intro loads are split
        # across both HWDGE dispatch engines (sync + scalar) since the
        # scalar engine is idle until the first exp.
        for mb in range(MB):
            ea = nc.scalar if mb % 2 else nc.sync
            eb = nc.sync if mb % 2 else nc.scalar
            ea.dma_start(wq_t[mb][:], d["wq"][128 * mb:128 * (mb + 1), :])
            eb.dma_start(xt_t[mb][:, 0:CH],
                         d["xt"][128 * mb:128 * (mb + 1), 0:CH])
            ea.dma_start(wk_t[mb][:], d["wk"][128 * mb:128 * (mb + 1), :])
        for mb in range(MB):
            eng = nc.sync if mb % 2 else nc.scalar
            eng.dma_start(wv_t[mb][:], d["wv"][128 * mb:128 * (mb + 1), :])
        if not zero_bias:
            nc.sync.dma_start(wq_b[:], d["wq"][1024:1025, :])
            nc.sync.dma_start(wk_b[:], d["wk"][1024:1025, :])
            nc.sync.dma_start(wv_b[:], d["wv"][1024:1025, :])
            nc.sync.dma_start(xt_ones[:], d["xt"][1024:1025, :])
        nc.scalar.dma_start(dtri[:], d["mk"][:])
        nc.sync.dma_start(ones_row[:], d["cst"][0:1, :])
        nc.sync.dma_start(ones_col[:], d["cst"][0:128, 0:1])
        nc.sync.dma_start(e2_t[:], d["e2"][:])
        for c in range(1, NCH):
            for mb in range(MB):
                nc.sync.dma_start(
                    xt_t[mb][:, CH * c:CH * (c + 1)],
                    d["xt"][128 * mb:128 * (mb + 1), CH * c:CH * (c + 1)])
        for p in range(NP):
            nc.sync.dma_start(wo_t[p][:], d["wo"][128 * p:128 * (p + 1), :])

        def emit_v(j):
            ps = psX.tile([128, 260], F32, name="psv", tag="px")
            for mb in range(MB):
                mm(ps[:], xt_t[mb][:, 128 * j:128 * (j + 1)],
                   wv_t[mb][:], start=(mb == 0),
                   stop=(zero_bias and mb == MB - 1))
            if not zero_bias:
                mm(ps[:], xt_ones[:, 128 * j:128 * (j + 1)], wv_b[:],
                   start=False, stop=True)
            nc.vector.tensor_copy(vt[j][:], ps[:])
            if zero_bias:
                # ones columns (the bias-row trick needs them even with zero
                # biases): set cols 64,129,194,259 to 1.0
                oc = vt[j].rearrange("p (h c) -> p h c", c=65)[:, :, 64]
                nc.gpsimd.memset(oc, 1.0)
            if j % 4:
                # per-block column sums of V_aug, for the fully-masked
                # column prefix of diagonal blocks
                bs = psX.tile([1, 260], F32, name="psb", tag="px")
                mm(bs[:], ones_col[:], vt[j][:], start=True, stop=True)
                nc.vector.tensor_copy(b_sb[j][:], bs[:])

        def emit_qk(p, which, c):
            dst, wt = (qt, wq_t) if which == 0 else (kt, wk_t)
            ps = psX.tile([128, CH], F32, name="psqk", tag="px")
            for mb in range(MB):
                mm(ps[:], wt[mb][:, 128 * p:128 * (p + 1)],
                   xt_t[mb][:, CH * c:CH * (c + 1)],
                   start=(mb == 0), stop=(zero_bias and mb == MB - 1))
            if not zero_bias:
                wb = wq_b if which == 0 else wk_b
                mm(ps[:], wb[:, 128 * p:128 * (p + 1)],
                   xt_ones[:, CH * c:CH * (c + 1)], start=False, stop=True)
            nc.vector.tensor_copy(dst[p][:, CH * c:CH * (c + 1)], ps[:])

        zps_by_chunk = {}

        def emit_attn_head(ch):
            # scores/mask/exp/AV for all k-blocks of chunk ch (both pairs)
            nj = 4 * ch + 4
            zl = []
            for p in range(NP):
                h0, h1 = 2 * p, 2 * p + 1
                zps = [psZ.tile([65, CH], F32, name=f"zps{half}",
                                tag=f"zps{half}") for half in range(2)]
                zl.append(zps)
                for j in range(nj):
                    # both heads of the pair share one [128,1024] scores
                    # psum: one mask-mul + one exp per j. For diagonal
                    # blocks only the unmasked column suffix [w0:512) is
                    # computed; the fully-masked prefix contributes
                    # exp(0)=1 per element, added analytically from the
                    # block's V column sums.
                    r = j - 4 * ch
                    w0 = 128 * r if r > 0 else 0
                    n = CH - w0
                    last = ch == 3 and j == nj - 1
                    sps = psS.tile([128, 2 * CH], F32, name="sps", tag="sps")
                    mm(sps[:, w0:CH],
                       kt[p][0:64, 128 * j:128 * (j + 1)],
                       qt[p][0:64, CH * ch + w0:CH * (ch + 1)],
                       start=True, stop=True)
                    mm(sps[:, CH + w0:2 * CH],
                       kt[p][64:128, 128 * j:128 * (j + 1)],
                       qt[p][64:128, CH * ch + w0:CH * (ch + 1)],
                       start=True, stop=True)
                    sps3 = sps.rearrange("p (t c) -> p t c", t=2)
                    pt = pp.tile([128, 2 * CH], BF16, name="pt", tag="pt")
                    pt3 = pt.rearrange("p (t c) -> p t c", t=2)
                    if r >= 0:
                        # triangular mask on the 128-wide diagonal strip of
                        # both heads at once
                        strip = sps3[:, :, w0:w0 + 128]
                        dtri3 = dtri.rearrange("p (t c) -> p t c", t=2)
                        nc.vector.tensor_mul(strip, strip, dtri3)
                    if w0:
                        nc.scalar.activation(pt3[:, :, w0:CH],
                                             sps3[:, :, w0:CH], EXP,
                                             scale=0.125)
                    else:
                        nc.scalar.activation(pt[:], sps[:], EXP, scale=0.125)
                    mm(zps[0][:, w0:CH], vt[j][:, 65 * h0:65 * h0 + 65],
                       pt[:, w0:CH], start=(j == 0),
                       stop=(last and not w0))
                    mm(zps[1][:, w0:CH], vt[j][:, 65 * h1:65 * h1 + 65],
                       pt[:, CH + w0:2 * CH], start=(j == 0),
                       stop=(last and not w0))
                    if w0:
                        mm(zps[0][:, 0:w0], b_sb[j][:, 65 * h0:65 * h0 + 65],
                           ones_row[:, 0:w0], start=False, stop=last)
                        mm(zps[1][:, 0:w0], b_sb[j][:, 65 * h1:65 * h1 + 65],
                           ones_row[:, 0:w0], start=False, stop=last)
            zps_by_chunk[ch] = zl

        def emit_attn_zrel(ch):
            # suffix contribution + denominator/z extraction for chunk ch —
            # releases the z psum slots as early as possible
            zl = zps_by_chunk.pop(ch)
            for p in range(NP):
                for half in range(2):
                    h = 2 * p + half
                    hb = 64 * half
                    if ch < 3:
                        mm(zl[p][half][:], sfx[ch][:, 65 * h:65 * h + 65],
                           ones_row[:], start=False, stop=True)
                    # stage denom row at partition 0, DMA-scatter into
                    # dnc[ch] (compute-engine APs need 32-aligned base
                    # partitions; DMA APs don't)
                    dnst = pp.tile([1, CH], F32, name="dnst", tag="dnst",
                                   bufs=4)
                    nc.vector.tensor_copy(dnst[:], zl[p][half][64:65, :])
                    nc.scalar.dma_start(dnc[ch][p][half:half + 1, :], dnst[:])
                    nc.vector.tensor_copy(
                        z2u[p][hb:hb + 64, CH * ch:CH * (ch + 1)],
                        zl[p][half][0:64, :])

        def emit_divE(ch):
            # divide chunk ch's z by the softmax denominators, project to
            # the output and stream to DRAM
            for p in range(NP):
                rdc = stg.tile([2, CH], F32, name="rdc", tag="rdc")
                nc.vector.reciprocal_approx_fast(rdc[:], dnc[ch][p][:])
                rdcb = stg.tile([2, CH], BF16, name="rdcb", tag="rdcb")
                nc.vector.tensor_copy(rdcb[:], rdc[:])
                bc = psX.tile([128, CH], F32, name="bc", tag="px")
                mm(bc[:], e2_t[:], rdcb[:], start=True, stop=True)
                nc.vector.tensor_mul(
                    z2u[p][:, CH * ch:CH * (ch + 1)],
                    z2u[p][:, CH * ch:CH * (ch + 1)], bc[:])

            for q in range(4 * ch, 4 * ch + 4):
                for mc in range(2):
                    ops = psX.tile([128, CH], F32, name="ops", tag="px")
                    for p in range(NP):
                        mm(ops[:], z2u[p][:, 128 * q:128 * (q + 1)],
                           wo_t[p][:, CH * mc:CH * (mc + 1)],
                           start=(p == 0), stop=(p == 1))
                    osb = op_sb.tile([128, CH], BF16, name="osb", tag="osb")
                    nc.vector.tensor_copy(osb[:], ops[:])
                    nc.sync.dma_start(
                        d["out"][128 * q:128 * (q + 1), CH * mc:CH * (mc + 1)],
                        osb[:])

        # ---- emission: interleave projections, attention, division and
        # output so every engine has work throughout. Chunk c's division +
        # output projection is emitted after chunk c+1's attention head so
        # the PE never blocks on the (latency-heavy) division chain. ----
        for p in range(NP):
            emit_qk(p, 0, 0)
            emit_qk(p, 1, 0)
        for j in range(4):
            emit_v(j)
        emit_attn_head(0)
        for j in range(4, KB):
            emit_v(j)
        # suffix column-sums of V_aug for fully-masked regions
        for c in range(3):
            ps = psX.tile([1, 260], F32, name="pssfx", tag="px")
            for j in range(4 * c + 4, KB):
                mm(ps[:], ones_col[:], vt[j][:],
                   start=(j == 4 * c + 4), stop=(j == KB - 1))
            nc.vector.tensor_copy(sfx[c][:], ps[:])
        emit_attn_zrel(0)
        for ch in range(1, NCH):
            for p in range(NP):
                emit_qk(p, 0, ch)
                emit_qk(p, 1, ch)
            emit_attn_head(ch)
            emit_divE(ch - 1)
            emit_attn_zrel(ch)
        emit_divE(3)


def build_program(zero_bias=False):
    nc = bacc.Bacc("TRN2", target_bir_lowering=False, debug=False,
                   num_devices=N_CORES)
    d = {
        "xt": nc.dram_tensor("xt", [1025, S], BF16, kind="ExternalInput").ap(),
        "wq": nc.dram_tensor("wq", [1025, 256], BF16, kind="ExternalInput").ap(),
        "wk": nc.dram_tensor("wk", [1025, 256], BF16, kind="ExternalInput").ap(),
        "wv": nc.dram_tensor("wv", [1025, 260], BF16, kind="ExternalInput").ap(),
        "wo": nc.dram_tensor("wo", [256, M], BF16, kind="ExternalInput").ap(),
        "mk": nc.dram_tensor("mk", [128, 256], BF16, kind="ExternalInput").ap(),
        "e2": nc.dram_tensor("e2", [2, 128], BF16, kind="ExternalInput").ap(),
        "cst": nc.dram_tensor("cst", [128, CH], BF16, kind="ExternalInput").ap(),
        "out": nc.dram_tensor("out", [S, M], BF16, kind="ExternalOutput").ap(),
    }
    with tile.TileContext(nc) as tc:
        _emit(tc, nc, d, zero_bias)
    nc.compile()
    return nc


_CACHE = {}


def _get_program(zero_bias=False):
    key = ("nc", zero_bias)
    if key not in _CACHE:
        _CACHE[key] = build_program(zero_bias)
    return _CACHE[key]


def _pack_qk(w4, b4):
    # w4 [4,1024,64], b4 [4,64] -> [1025, 256] (m-major, head-major cols)
    r = np.empty((1025, 256), np.float32)
    r[:1024] = w4.transpose(1, 0, 2).reshape(1024, 256)
    r[1024] = b4.reshape(256)
    return r


def _pack_v(w4, b4):
    # [1025, 260]: per head 64 W_V cols + a ones-generating column
    r = np.zeros((1025, 260), np.float32)
    for h in range(4):
        r[:1024, 65 * h:65 * h + 64] = w4[h]
        r[1024, 65 * h:65 * h + 64] = b4[h]
        r[1024, 65 * h + 64] = 1.0
    return r


def prepare_in_maps(normalized_resid_pre, W_Q, b_Q, W_K, b_K, W_V, b_V, W_O,
                    b_O):
    import ml_dtypes
    bf16 = ml_dtypes.bfloat16
    x = np.asarray(normalized_resid_pre, np.float32)
    W_Q = np.asarray(W_Q, np.float32)
    b_Q = np.asarray(b_Q, np.float32)
    W_K = np.asarray(W_K, np.float32)
    b_K = np.asarray(b_K, np.float32)
    W_V = np.asarray(W_V, np.float32)
    b_V = np.asarray(b_V, np.float32)
    W_O = np.asarray(W_O, np.float32)

    tri = np.triu(np.ones((128, 128), np.float32))  # [k,q]: 1 where k <= q
    mk = np.tile(tri, (1, 2))  # both heads of a pair side by side
    e2 = np.zeros((2, 128), np.float32)
    e2[0, :64] = 1.0
    e2[1, 64:] = 1.0
    cstv = np.ones((128, CH), np.float32)

    xts = []
    for b in range(2):
        xt = np.empty((1025, S), np.float32)
        xt[:1024] = x[b].T
        xt[1024] = 1.0
        xts.append(xt.astype(bf16))

    in_maps = []
    for c in range(N_CORES):
        b, g = divmod(c, 4)
        hs = slice(4 * g, 4 * g + 4)
        in_maps.append({
            "xt": xts[b],
            "wq": _pack_qk(W_Q[hs], b_Q[hs]).astype(bf16),
            "wk": _pack_qk(W_K[hs], b_K[hs]).astype(bf16),
            "wv": _pack_v(W_V[hs], b_V[hs]).astype(bf16),
            "wo": np.ascontiguousarray(W_O[hs].reshape(256, M)).astype(bf16),
            "mk": mk.astype(bf16),
            "e2": e2.astype(bf16),
            "cst": cstv.astype(bf16),
        })
    return in_maps


def gather(results, b_O):
    out = np.zeros((2, S, M), np.float32)
    for c in range(N_CORES):
        out[c // 4] += np.asarray(results[c]["out"], dtype=np.float32)
    out += np.asarray(b_O, np.float32)[None, None, :]
    return out


def _run(in_maps, trace=False, zero_bias=False, **kw):
    nc = _get_program(zero_bias)
    return bass_utils.run_bass_kernel_spmd(
        nc, in_maps, core_ids=list(range(N_CORES)), trace=trace, **kw)


def all_zero_bias(b_Q, b_K, b_V):
    return (not np.any(np.asarray(b_Q)) and not np.any(np.asarray(b_K))
            and not np.any(np.asarray(b_V)))


def kernel(normalized_resid_pre, W_Q, b_Q, W_K, b_K, W_V, b_V, W_O, b_O):
    in_maps = prepare_in_maps(normalized_resid_pre, W_Q, b_Q, W_K, b_K, W_V,
                              b_V, W_O, b_O)
    res = _run(in_maps, zero_bias=all_zero_bias(b_Q, b_K, b_V))
    return gather(res.results, b_O)
